# revision 15
# baseline (speedup 1.0000x reference)
"""Varlen causal GQA flash attention on 8 TRN2 NeuronCores.

Sharding: tensor-parallel over heads. Core i gets Q heads [4i, 4i+4) and
KV head i (GQA group kept intact) -> zero cross-core communication.

Per-core kernel (specialized at build time on the host-visible cu_seqlens):
for each packed sequence (start, L) and each 128-wide query block qb:
  - S^T matmul: lhsT = K^T chunk [128d, <=128 keys], rhs = Q^T [128d, 4h*Lq]
    -> PSUM S^T [keys, (h,q)] (N = 512 streams, bf16).
  - exp on ScalarE straight out of PSUM -> bf16 P^T in SBUF (scale folded in).
  - causal mask on the diagonal chunk: multiply by a 0/1 upper-tri mask (DVE).
  - PV matmuls: lhsT = V chunk [keys, 128d] (natural layout, no P transpose),
    rhs = P^T -> accumulate O^T [128d, 4h*Lq] in PSUM.
  - denominator: all-ones [keys,128] lhsT matmul -> row sums replicated
    across all 128 partitions in PSUM; reciprocal_approx_fast; one
    tensor_tensor multiply normalizes O^T on the way PSUM->SBUF.
  - DMA O^T out; host undoes the transposes (host layout work is free).

The whole core's work is emitted as one flat software pipeline over
(seq, qb, chunk-group) tasks with the S^T matmuls running two groups ahead
of the PV/SUM consumers, so the PE never sits behind exp/mask latency.
"""

import math
import os
import sys

import numpy as np

for _p in ("/opt/trn_rl_repo", "/root/.axon_site/_ro/trn_rl_repo"):
    if os.path.isdir(_p) and _p not in sys.path:
        sys.path.append(_p)

# Under an axon-tunneled container the device run goes through the jax "axon"
# platform; make sure an explicit JAX_PLATFORMS=cpu doesn't hide the devices.
if os.environ.get("TRN_TERMINAL_POOL_IPS") and "jax" not in sys.modules:
    _jp = os.environ.get("JAX_PLATFORMS", "")
    if _jp and "axon" not in _jp:
        os.environ["JAX_PLATFORMS"] = "axon," + _jp

import ml_dtypes

import concourse.bass as bass
import concourse.mybir as mybir
import concourse.tile as tile
from concourse import bacc
from concourse.bass_utils import run_bass_kernel_spmd
from concourse.masks import make_upper_triangular

NUM_HEADS = 32
NUM_KV_HEADS = 8
HEAD_DIM = 128
SCALE = 1.0 / float(np.sqrt(HEAD_DIM))
MAX_SEQLEN = 1024
NUM_SEQS = 4
T_TOTAL = NUM_SEQS * MAX_SEQLEN
N_CORES = 8
HPC = NUM_HEADS // N_CORES  # q heads per core = 4
BF16 = ml_dtypes.bfloat16
GROUP = 2  # key chunks per exp group (PSUM-bank budget bound)

_GRAPH_CACHE = {}


def build_graph(Ls, lookahead=2):
    """Build the SPMD Bass graph, specialized on per-sequence lengths Ls."""
    DT = mybir.dt.bfloat16
    F32 = mybir.dt.float32
    nc = bacc.Bacc(
        "TRN2",
        target_bir_lowering=False,
        debug=False,
        enable_asserts=False,
        num_devices=N_CORES,
    )
    qT = nc.dram_tensor("qT", [NUM_SEQS, 128, HPC, MAX_SEQLEN], DT, kind="ExternalInput")
    kT = nc.dram_tensor("kT", [128, NUM_SEQS, MAX_SEQLEN], DT, kind="ExternalInput")
    vv = nc.dram_tensor("vv", [128, NUM_SEQS, MAX_SEQLEN // 128, 128], DT, kind="ExternalInput")
    outT = nc.dram_tensor("out", [128, HPC, NUM_SEQS, MAX_SEQLEN], DT, kind="ExternalOutput")

    mult = mybir.AluOpType.mult
    active = [(s, L) for s, L in enumerate(Ls) if L > 0]
    nact = len(active)

    with tile.TileContext(nc) as tc:
        with (
            tc.tile_pool(name="consts", bufs=1) as consts,
            tc.tile_pool(name="kin", bufs=nact) as kin,
            tc.tile_pool(name="vin", bufs=nact) as vin,
            tc.tile_pool(name="qin", bufs=nact) as qin,
            tc.tile_pool(name="pt", bufs=8) as ppool,
            tc.tile_pool(name="pairp", bufs=7) as pairp,
            tc.tile_pool(name="osb", bufs=6) as osb,
            tc.tile_pool(name="invp", bufs=3) as invp,
            tc.tile_pool(name="spsum", bufs=2, space="PSUM") as spsum,
            tc.tile_pool(name="opsum", bufs=2, space="PSUM") as opsum,
            tc.tile_pool(name="smpsum", bufs=2, space="PSUM") as smpsum,
        ):
            mask1 = consts.tile([128, 128], DT)
            make_upper_triangular(nc, mask1[:], val=1.0, diag=True)
            # per-head copy keeps the mask multiply off the slow broadcast path
            mask = consts.tile([128, HPC, 128], DT)
            for h in range(HPC):
                nc.vector.tensor_copy(mask[:, h, :], mask1[:])
            ones = consts.tile([128, 128], DT)
            nc.vector.memset(ones[:], 1.0)
            sbufs = {}
            for s, L in active:
                nqb = math.ceil(L / 128)
                k_sb = kin.tile([128, MAX_SEQLEN], DT, tag="k", name=f"k_{s}")
                v_sb = vin.tile([128, MAX_SEQLEN // 128, 128], DT, tag="v", name=f"v_{s}")
                q_sb = qin.tile([128, HPC, MAX_SEQLEN], DT, tag="q", name=f"q_{s}")
                sbufs[s] = (k_sb, v_sb, q_sb, nqb)
            # ALL input DMAs ride the sync queue, finest pieces first, so the
            # scalar queue carries nothing but the act-table warm + the exp
            # stream, and the early tasks are never gated on bulk transfers.
            warm = consts.tile([128, 1], F32)
            first = True
            for s, L in active:
                k_sb, v_sb, q_sb, nqb = sbufs[s]
                if first:
                    nc.sync.dma_start(k_sb[:, : min(128, L)], kT[:, s, : min(128, L)])
                    nc.sync.dma_start(q_sb[:, :, : min(128, L)], qT[s, :, :, : min(128, L)])
                    nc.sync.dma_start(v_sb[:, :1, :], vv[:, s, :1, :])
                    # warm the exp table while the first pieces are in flight
                    nc.scalar.activation(
                        warm[:], mask1[:, :1], mybir.ActivationFunctionType.Exp, scale=0.0
                    )
                    if L > 128:
                        nc.sync.dma_start(k_sb[:, 128 : min(256, L)], kT[:, s, 128 : min(256, L)])
                        nc.sync.dma_start(q_sb[:, :, 128 : min(256, L)], qT[s, :, :, 128 : min(256, L)])
                    if L > 256:
                        nc.sync.dma_start(k_sb[:, 256:L], kT[:, s, 256:L])
                        nc.sync.dma_start(q_sb[:, :, 256 : min(384, L)], qT[s, :, :, 256 : min(384, L)])
                    if nqb > 1:
                        nc.sync.dma_start(v_sb[:, 1:nqb, :], vv[:, s, 1:nqb, :])
                    for a, b in ((384, 512), (512, 768), (768, MAX_SEQLEN)):
                        if L > a:
                            nc.sync.dma_start(q_sb[:, :, a : min(b, L)], qT[s, :, :, a : min(b, L)])
                    first = False
                else:
                    nc.sync.dma_start(k_sb[:, :L], kT[:, s, :L])
                    nc.sync.dma_start(q_sb[:, :, : min(512, L)], qT[s, :, :, : min(512, L)])
                    if L > 512:
                        nc.sync.dma_start(q_sb[:, :, 512:L], qT[s, :, :, 512:L])
                    nc.sync.dma_start(v_sb[:, :nqb, :], vv[:, s, :nqb, :])

            # ---- flat task list: one task per (seq, qb, chunk-group)
            # chunks within a qb run diagonal-first (reverse order).
            tasks = []
            for s, L in active:
                nqb = sbufs[s][3]
                for qb in range(nqb):
                    order = list(range(qb, -1, -1))
                    groups = [order[g : g + GROUP] for g in range(0, len(order), GROUP)]
                    for gi, cg in enumerate(groups):
                        tasks.append((s, L, qb, gi, cg, gi == len(groups) - 1))
            i = 1
            while i < len(tasks):
                if tasks[i][0] != tasks[i - 1][0]:
                    tasks[i - 1], tasks[i] = tasks[i], tasks[i - 1]
                    i += 2
                else:
                    i += 1

            # Row styles. qb0: everything inline. qb1-3 ("defer"): per-chunk
            # ones-matmuls, only the diagonal deferred one task. qb>=4
            # ("tree"): denominator P chunks reduced off the PE - independent
            # pair-adds on GPSIMD, cheap merges on DVE (deferred one task so
            # DVE never head-blocks on a GPSIMD result), one ones-matmul per
            # row. Masks: qb<=1 DVE, qb>=2 GPSIMD (diag PV deferral hides the
            # slow GPSIMD op behind the row).
            def style_of(qb, nqb):
                if qb == 0:
                    return "simple"
                if qb >= 4 and qb >= nqb - 4:
                    return "tree"
                return "defer"

            s_tiles = {}

            def emit_S(t):
                s, L, qb, gi, cg, _last = tasks[t]
                _, _, q_sb, _ = sbufs[s]
                k_sb = sbufs[s][0]
                Lq = min(128, L - qb * 128)
                qs = q_sb[:, :, qb * 128 : qb * 128 + Lq]
                st = spsum.tile([128, GROUP, HPC, 128], F32, tag="s")
                s_tiles[t] = st
                for ci, c in enumerate(cg):
                    Lk = min(128, L - c * 128)
                    nc.tensor.matmul(
                        st[:Lk, ci, :, :Lq],
                        lhsT=k_sb[:, c * 128 : c * 128 + Lk],
                        rhs=qs,
                        start=True,
                        stop=True,
                    )

            # row state: [o_ps, sum_ps, pv_started, sum_started, acc, diag]
            cur = {}
            o_tiles = {}
            sum_q = []    # deferred SUM-matmul jobs: (s, qb, rhs_ap, Lk, Lq, last)
            epi_q = []    # epilogues (recip+normalize), deferred
            merge_q = []  # DVE merge jobs (s, qb, part_ap), deferred one task
            fin_q = []    # row-finish jobs (s, qb), deferred one task

            def epilogue(s_, qb_):
                L_ = dict(active)[s_]
                nqb_ = sbufs[s_][3]
                Lq_ = min(128, L_ - qb_ * 128)
                row = cur.pop((s_, qb_))
                o_ps, sum_ps = row[0], row[1]
                inv = invp.tile([128, HPC, 128], F32, tag="inv", name=f"inv_{s_}_{qb_}")
                nc.vector.reciprocal_approx_fast(inv[:, :, :Lq_], sum_ps[:, :, :Lq_])
                if qb_ % 2 == 0:
                    o_tiles[s_] = osb.tile([128, HPC, 256], DT, tag="ot", name=f"ot_{s_}_{qb_}")
                o_tile = o_tiles[s_]
                slot = (qb_ % 2) * 128
                nc.vector.tensor_tensor(
                    o_tile[:, :, slot : slot + Lq_], o_ps[:, :, :Lq_], inv[:, :, :Lq_], mult
                )
                if qb_ % 2 == 1 or qb_ == nqb_ - 1:
                    t0 = (qb_ - (qb_ % 2)) * 128
                    w = (qb_ % 2) * 128 + Lq_
                    nc.sync.dma_start(outT[:, :, s_, t0 : t0 + w], o_tile[:, :, :w])

            def drain_sums(keep):
                while len(sum_q) > keep:
                    s_, qb_, rhs, Lk_, Lq_, last_ = sum_q.pop(0)
                    row = cur[(s_, qb_)]
                    nc.tensor.matmul(
                        row[1][:, :, :Lq_],
                        lhsT=ones[:Lk_, :],
                        rhs=rhs,
                        start=(not row[3]),
                        stop=last_,
                    )
                    row[3] = True
                    if last_:
                        epi_q.append((s_, qb_))

            def emit_pv(row, v_sb_, c, p_ap, Lk, Lq, stop):
                nc.tensor.matmul(
                    row[0][:, :, :Lq],
                    lhsT=v_sb_[:Lk, c, :],
                    rhs=p_ap,
                    start=(not row[2]),
                    stop=stop,
                )
                row[2] = True

            def drain_merges():
                while merge_q:
                    s_, qb_, part = merge_q.pop(0)
                    L_ = dict(active)[s_]
                    Lq_ = min(128, L_ - qb_ * 128)
                    row = cur[(s_, qb_)]
                    if row[4] is None:
                        row[4] = ("part", part)
                    elif row[4][0] == "part":
                        acc = pairp.tile([128, HPC, 128], DT, tag="pp")
                        nc.vector.tensor_add(acc[:, :, :Lq_], row[4][1], part)
                        row[4] = ("acc", acc)
                    else:
                        acc = row[4][1]
                        nc.vector.tensor_add(acc[:, :, :Lq_], acc[:, :, :Lq_], part)

            def finish_row(s_, qb_):
                L_ = dict(active)[s_]
                Lq_ = min(128, L_ - qb_ * 128)
                row = cur[(s_, qb_)]
                v_sb_ = sbufs[s_][1]
                dp, dLk = row[5]
                emit_pv(row, v_sb_, qb_, dp, dLk, Lq_, stop=True)
                if row[4] is not None:  # tree row: fold diag into acc, one SUM
                    if row[4][0] == "part":
                        acc = pairp.tile([128, HPC, 128], DT, tag="pp")
                        nc.vector.tensor_add(acc[:dLk, :, :Lq_], row[4][1][:dLk], dp)
                        if dLk < 128:
                            nc.vector.tensor_copy(acc[dLk:, :, :Lq_], row[4][1][dLk:])
                        row[4] = ("acc", acc)
                    else:
                        acc = row[4][1]
                        nc.vector.tensor_add(acc[:dLk, :, :Lq_], acc[:dLk, :, :Lq_], dp)
                    sum_q.append((s_, qb_, row[4][1][:, :, :Lq_], 128, Lq_, True))
                else:  # defer row: diag keeps its own ones-matmul
                    sum_q.append((s_, qb_, dp, dLk, Lq_, True))

            for t in range(min(lookahead, len(tasks))):
                emit_S(t)
            for t, (s, L, qb, gi, cg, last) in enumerate(tasks):
                if t + lookahead < len(tasks):
                    emit_S(t + lookahead)
                k_sb, v_sb, q_sb, nqb = sbufs[s]
                style = style_of(qb, nqb)
                Lq = min(128, L - qb * 128)
                st = s_tiles.pop(t)
                pt = ppool.tile([128, GROUP, HPC, 128], DT, tag="p")
                nc.scalar.activation(
                    pt[:, : len(cg), :, :Lq],
                    st[:, : len(cg), :, :Lq],
                    mybir.ActivationFunctionType.Exp,
                    scale=SCALE,
                )
                drain_merges()
                while fin_q:
                    finish_row(*fin_q.pop(0))
                if cg[0] == qb and gi == 0:  # diagonal chunk: causal 0/1 mask
                    meng = nc.vector if qb <= 1 else nc.gpsimd
                    meng.tensor_tensor(
                        pt[:Lq, 0, :, :Lq],
                        pt[:Lq, 0, :, :Lq],
                        mask[:Lq, :, :Lq],
                        mult,
                    )
                while epi_q:
                    epilogue(*epi_q.pop(0))
                if gi == 0:
                    o_ps = opsum.tile([128, HPC, 128], F32, tag="o", name=f"o_{s}_{qb}")
                    sum_ps = smpsum.tile([128, HPC, 128], F32, tag="sm", name=f"sm_{s}_{qb}")
                    cur[(s, qb)] = [o_ps, sum_ps, False, False, None, None]
                row = cur[(s, qb)]
                tree = style == "tree"
                grp_parts = []
                for ci, c in enumerate(cg):
                    Lk = min(128, L - c * 128)
                    p_ap = pt[:Lk, ci, :, :Lq]
                    if c == qb and style != "simple":
                        row[5] = (p_ap, Lk)  # defer diag PV + its sum
                        continue
                    emit_pv(row, v_sb, c, p_ap, Lk, Lq,
                            stop=(style == "simple" and last and ci == len(cg) - 1))
                    if tree:
                        grp_parts.append((p_ap, Lk))
                    else:
                        sum_q.append((s, qb, p_ap, Lk, Lq, style == "simple"
                                      and last and ci == len(cg) - 1))
                if tree and grp_parts:
                    if len(grp_parts) == 2 and grp_parts[0][1] == 128 and grp_parts[1][1] == 128:
                        # independent pair-add on the idle GPSIMD
                        pa = pairp.tile([128, HPC, 128], DT, tag="pp")
                        nc.gpsimd.tensor_add(pa[:, :, :Lq], grp_parts[0][0], grp_parts[1][0])
                        merge_q.append((s, qb, pa[:, :, :Lq]))
                    else:
                        for p_ap, _ in grp_parts:
                            merge_q.append((s, qb, p_ap))
                if last:
                    if style == "simple":
                        pass
                    else:
                        fin_q.append((s, qb))
                drain_sums(1)
            drain_merges()
            while fin_q:
                finish_row(*fin_q.pop(0))
            drain_sums(0)
            while epi_q:
                epilogue(*epi_q.pop(0))
    nc.compile()
    return nc


def get_graph(Ls):
    key = tuple(Ls)
    if key not in _GRAPH_CACHE:
        _GRAPH_CACHE[key] = build_graph(key)
    return _GRAPH_CACHE[key]


def _prep_shards(q, k, v, seqs):
    """Host-side shard + pad + transpose. Returns in_maps for the 8 cores."""
    qb = q.astype(BF16)
    kb = k.astype(BF16)
    vb = v.astype(BF16)
    qp = np.zeros((NUM_SEQS, MAX_SEQLEN, NUM_HEADS, HEAD_DIM), dtype=BF16)
    kp = np.zeros((NUM_SEQS, MAX_SEQLEN, NUM_KV_HEADS, HEAD_DIM), dtype=BF16)
    vp = np.zeros((NUM_SEQS, MAX_SEQLEN, NUM_KV_HEADS, HEAD_DIM), dtype=BF16)
    for s, (st, L) in enumerate(seqs):
        if L:
            qp[s, :L] = qb[st : st + L]
            kp[s, :L] = kb[st : st + L]
            vp[s, :L] = vb[st : st + L]
    in_maps = []
    for i in range(N_CORES):
        hs = slice(HPC * i, HPC * (i + 1))
        qTa = np.ascontiguousarray(qp[:, :, hs, :].transpose(0, 3, 2, 1))
        kTa = np.ascontiguousarray(kp[:, :, i, :].transpose(2, 0, 1))
        vva = np.ascontiguousarray(
            vp[:, :, i, :].reshape(NUM_SEQS, MAX_SEQLEN // 128, 128, HEAD_DIM).transpose(2, 0, 1, 3)
        )
        in_maps.append({"qT": qTa, "kT": kTa, "vv": vva})
    return in_maps


def kernel(q, k, v, cu_seqlens, _trace=False, _tmpdir=None):
    q = np.asarray(q)
    k = np.asarray(k)
    v = np.asarray(v)
    cu = np.asarray(cu_seqlens).astype(np.int64)
    starts = cu[:-1]
    lens = np.clip(cu[1:] - cu[:-1], 0, MAX_SEQLEN)
    seqs = [(int(starts[b]), int(lens[b])) for b in range(NUM_SEQS)]

    out = np.zeros((T_TOTAL, NUM_HEADS, HEAD_DIM), dtype=q.dtype)
    if all(L == 0 for _, L in seqs):
        return out

    nc = get_graph([L for _, L in seqs])
    in_maps = _prep_shards(q, k, v, seqs)
    res = run_bass_kernel_spmd(
        nc,
        in_maps,
        core_ids=list(range(N_CORES)),
        trace=_trace,
        tmpdir=_tmpdir,
    )
    for i in range(N_CORES):
        oT = res.results[i]["out"]  # [128 d, 4 h, s, t] bf16
        o = oT.astype(np.float32).transpose(2, 3, 1, 0)  # [s, t, h, d]
        for s, (st, L) in enumerate(seqs):
            if L:
                out[st : st + L, HPC * i : HPC * (i + 1), :] = o[s, :L]
    if _trace:
        return out, res
    return out



# revision 16
# speedup vs baseline: 1.2909x; 1.2909x over previous
"""Varlen causal GQA flash attention on 8 TRN2 NeuronCores.

Sharding: tensor-parallel over heads. Core i gets Q heads [4i, 4i+4) and
KV head i (GQA group kept intact) -> zero cross-core communication.

Per-core kernel (specialized at build time on the host-visible cu_seqlens):
for each packed sequence (start, L) and each 128-wide query block qb:
  - S^T matmul: lhsT = K^T chunk [128d, <=128 keys], rhs = Q^T [128d, 4h*Lq]
    -> PSUM S^T [keys, (h,q)] (N = 512 streams, bf16).
  - exp on ScalarE straight out of PSUM -> bf16 P^T in SBUF (scale folded in).
  - causal mask on the diagonal chunk: multiply by a 0/1 upper-tri mask (DVE).
  - PV matmuls: lhsT = V chunk [keys, 128d] (natural layout, no P transpose),
    rhs = P^T -> accumulate O^T [128d, 4h*Lq] in PSUM.
  - denominator: all-ones [keys,128] lhsT matmul -> row sums replicated
    across all 128 partitions in PSUM; reciprocal_approx_fast; one
    tensor_tensor multiply normalizes O^T on the way PSUM->SBUF.
  - DMA O^T out; host undoes the transposes (host layout work is free).

The whole core's work is emitted as one flat software pipeline over
(seq, qb, chunk-group) tasks with the S^T matmuls running two groups ahead
of the PV/SUM consumers, so the PE never sits behind exp/mask latency.
"""

import math
import os
import sys

import numpy as np

for _p in ("/opt/trn_rl_repo", "/root/.axon_site/_ro/trn_rl_repo"):
    if os.path.isdir(_p) and _p not in sys.path:
        sys.path.append(_p)

# Under an axon-tunneled container the device run goes through the jax "axon"
# platform; make sure an explicit JAX_PLATFORMS=cpu doesn't hide the devices.
if os.environ.get("TRN_TERMINAL_POOL_IPS") and "jax" not in sys.modules:
    _jp = os.environ.get("JAX_PLATFORMS", "")
    if _jp and "axon" not in _jp:
        os.environ["JAX_PLATFORMS"] = "axon," + _jp

import ml_dtypes

import concourse.bass as bass
import concourse.mybir as mybir
import concourse.tile as tile
from concourse import bacc
from concourse.bass_utils import run_bass_kernel_spmd
from concourse.masks import make_upper_triangular

NUM_HEADS = 32
NUM_KV_HEADS = 8
HEAD_DIM = 128
SCALE = 1.0 / float(np.sqrt(HEAD_DIM))
MAX_SEQLEN = 1024
NUM_SEQS = 4
T_TOTAL = NUM_SEQS * MAX_SEQLEN
N_CORES = 8
HPC = NUM_HEADS // N_CORES  # q heads per core = 4
BF16 = ml_dtypes.bfloat16
GROUP = 2  # key chunks per exp group (PSUM-bank budget bound)

_GRAPH_CACHE = {}


def build_graph(Ls, lookahead=2):
    """Build the SPMD Bass graph, specialized on per-sequence lengths Ls."""
    DT = mybir.dt.bfloat16
    F32 = mybir.dt.float32
    nc = bacc.Bacc(
        "TRN2",
        target_bir_lowering=False,
        debug=False,
        enable_asserts=False,
        num_devices=N_CORES,
    )
    qT = nc.dram_tensor("qT", [NUM_SEQS, 128, HPC, MAX_SEQLEN], DT, kind="ExternalInput")
    kT = nc.dram_tensor("kT", [128, NUM_SEQS, MAX_SEQLEN], DT, kind="ExternalInput")
    vv = nc.dram_tensor("vv", [128, NUM_SEQS, MAX_SEQLEN // 128, 128], DT, kind="ExternalInput")
    outT = nc.dram_tensor("out", [128, HPC, NUM_SEQS, MAX_SEQLEN], DT, kind="ExternalOutput")

    mult = mybir.AluOpType.mult
    active = [(s, L) for s, L in enumerate(Ls) if L > 0]
    nact = len(active)

    with tile.TileContext(nc) as tc:
        with (
            tc.tile_pool(name="consts", bufs=1) as consts,
            tc.tile_pool(name="kin", bufs=nact) as kin,
            tc.tile_pool(name="vin", bufs=nact) as vin,
            tc.tile_pool(name="qin", bufs=nact) as qin,
            tc.tile_pool(name="pt", bufs=8) as ppool,
            tc.tile_pool(name="pairp", bufs=7) as pairp,
            tc.tile_pool(name="osb", bufs=6) as osb,
            tc.tile_pool(name="invp", bufs=3) as invp,
            tc.tile_pool(name="spsum", bufs=2, space="PSUM") as spsum,
            tc.tile_pool(name="opsum", bufs=2, space="PSUM") as opsum,
            tc.tile_pool(name="smpsum", bufs=2, space="PSUM") as smpsum,
        ):
            mask1 = consts.tile([128, 128], DT)
            make_upper_triangular(nc, mask1[:], val=1.0, diag=True)
            # per-head copy keeps the mask multiply off the slow broadcast path
            mask = consts.tile([128, HPC, 128], DT)
            for h in range(HPC):
                nc.vector.tensor_copy(mask[:, h, :], mask1[:])
            ones = consts.tile([128, 128], DT)
            nc.vector.memset(ones[:], 1.0)
            sbufs = {}
            for s, L in active:
                nqb = math.ceil(L / 128)
                k_sb = kin.tile([128, MAX_SEQLEN], DT, tag="k", name=f"k_{s}")
                v_sb = vin.tile([128, MAX_SEQLEN // 128, 128], DT, tag="v", name=f"v_{s}")
                q_sb = qin.tile([128, HPC, MAX_SEQLEN], DT, tag="q", name=f"q_{s}")
                sbufs[s] = (k_sb, v_sb, q_sb, nqb)
            # ALL input DMAs ride the sync queue, finest pieces first, so the
            # scalar queue carries nothing but the act-table warm + the exp
            # stream, and the early tasks are never gated on bulk transfers.
            warm = consts.tile([128, 1], F32)
            first = True
            for s, L in active:
                k_sb, v_sb, q_sb, nqb = sbufs[s]
                if first:
                    nc.sync.dma_start(k_sb[:, : min(128, L)], kT[:, s, : min(128, L)])
                    nc.sync.dma_start(q_sb[:, :, : min(128, L)], qT[s, :, :, : min(128, L)])
                    nc.sync.dma_start(v_sb[:, :1, :], vv[:, s, :1, :])
                    # warm the exp table while the first pieces are in flight
                    nc.scalar.activation(
                        warm[:], mask1[:, :1], mybir.ActivationFunctionType.Exp, scale=0.0
                    )
                    if L > 128:
                        nc.sync.dma_start(k_sb[:, 128 : min(256, L)], kT[:, s, 128 : min(256, L)])
                        nc.sync.dma_start(q_sb[:, :, 128 : min(256, L)], qT[s, :, :, 128 : min(256, L)])
                    if L > 256:
                        nc.sync.dma_start(k_sb[:, 256:L], kT[:, s, 256:L])
                        nc.sync.dma_start(q_sb[:, :, 256 : min(384, L)], qT[s, :, :, 256 : min(384, L)])
                    if nqb > 1:
                        nc.sync.dma_start(v_sb[:, 1:nqb, :], vv[:, s, 1:nqb, :])
                    for a, b in ((384, 512), (512, 768), (768, MAX_SEQLEN)):
                        if L > a:
                            nc.sync.dma_start(q_sb[:, :, a : min(b, L)], qT[s, :, :, a : min(b, L)])
                    first = False
                else:
                    nc.sync.dma_start(k_sb[:, :L], kT[:, s, :L])
                    nc.sync.dma_start(q_sb[:, :, : min(512, L)], qT[s, :, :, : min(512, L)])
                    if L > 512:
                        nc.sync.dma_start(q_sb[:, :, 512:L], qT[s, :, :, 512:L])
                    nc.sync.dma_start(v_sb[:, :nqb, :], vv[:, s, :nqb, :])

            # ---- flat task list: one task per (seq, qb, chunk-group)
            # chunks within a qb run diagonal-first (reverse order).
            tasks = []
            for s, L in active:
                nqb = sbufs[s][3]
                for qb in range(nqb):
                    order = list(range(qb, -1, -1))
                    groups = [order[g : g + GROUP] for g in range(0, len(order), GROUP)]
                    for gi, cg in enumerate(groups):
                        tasks.append((s, L, qb, gi, cg, gi == len(groups) - 1))
            i = 1
            while i < len(tasks):
                if tasks[i][0] != tasks[i - 1][0]:
                    tasks[i - 1], tasks[i] = tasks[i], tasks[i - 1]
                    i += 2
                else:
                    i += 1

            # Row styles. qb0: everything inline. qb1-3 ("defer"): per-chunk
            # ones-matmuls, only the diagonal deferred one task. qb>=4
            # ("tree"): denominator P chunks reduced off the PE - independent
            # pair-adds on GPSIMD, cheap merges on DVE (deferred one task so
            # DVE never head-blocks on a GPSIMD result), one ones-matmul per
            # row. Masks: qb<=1 DVE, qb>=2 GPSIMD (diag PV deferral hides the
            # slow GPSIMD op behind the row).
            def style_of(qb, nqb):
                if qb == 0:
                    return "simple"
                if qb >= 4 and qb >= nqb - 2:
                    return "tree"
                return "defer"

            s_tiles = {}

            def emit_S(t):
                s, L, qb, gi, cg, _last = tasks[t]
                _, _, q_sb, _ = sbufs[s]
                k_sb = sbufs[s][0]
                Lq = min(128, L - qb * 128)
                qs = q_sb[:, :, qb * 128 : qb * 128 + Lq]
                st = spsum.tile([128, GROUP, HPC, 128], F32, tag="s")
                s_tiles[t] = st
                for ci, c in enumerate(cg):
                    Lk = min(128, L - c * 128)
                    nc.tensor.matmul(
                        st[:Lk, ci, :, :Lq],
                        lhsT=k_sb[:, c * 128 : c * 128 + Lk],
                        rhs=qs,
                        start=True,
                        stop=True,
                    )

            # row state: [o_ps, sum_ps, pv_started, sum_started, acc, diag]
            cur = {}
            o_tiles = {}
            sum_q = []    # deferred SUM-matmul jobs: (s, qb, rhs_ap, Lk, Lq, last)
            epi_q = []    # epilogues (recip+normalize), deferred
            merge_q = []  # DVE merge jobs (s, qb, part_ap), deferred one task
            fin_q = []    # row-finish jobs (s, qb), deferred one task

            def epilogue(s_, qb_):
                L_ = dict(active)[s_]
                nqb_ = sbufs[s_][3]
                Lq_ = min(128, L_ - qb_ * 128)
                row = cur.pop((s_, qb_))
                o_ps, sum_ps = row[0], row[1]
                inv = invp.tile([128, HPC, 128], F32, tag="inv", name=f"inv_{s_}_{qb_}")
                nc.vector.reciprocal_approx_fast(inv[:, :, :Lq_], sum_ps[:, :, :Lq_])
                if qb_ % 2 == 0:
                    o_tiles[s_] = osb.tile([128, HPC, 256], DT, tag="ot", name=f"ot_{s_}_{qb_}")
                o_tile = o_tiles[s_]
                slot = (qb_ % 2) * 128
                nc.vector.tensor_tensor(
                    o_tile[:, :, slot : slot + Lq_], o_ps[:, :, :Lq_], inv[:, :, :Lq_], mult
                )
                if qb_ % 2 == 1 or qb_ == nqb_ - 1:
                    t0 = (qb_ - (qb_ % 2)) * 128
                    w = (qb_ % 2) * 128 + Lq_
                    nc.sync.dma_start(outT[:, :, s_, t0 : t0 + w], o_tile[:, :, :w])

            def drain_sums(keep):
                while len(sum_q) > keep:
                    s_, qb_, rhs, Lk_, Lq_, last_ = sum_q.pop(0)
                    row = cur[(s_, qb_)]
                    nc.tensor.matmul(
                        row[1][:, :, :Lq_],
                        lhsT=ones[:Lk_, :],
                        rhs=rhs,
                        start=(not row[3]),
                        stop=last_,
                    )
                    row[3] = True
                    if last_:
                        epi_q.append((s_, qb_))

            def emit_pv(row, v_sb_, c, p_ap, Lk, Lq, stop):
                nc.tensor.matmul(
                    row[0][:, :, :Lq],
                    lhsT=v_sb_[:Lk, c, :],
                    rhs=p_ap,
                    start=(not row[2]),
                    stop=stop,
                )
                row[2] = True

            def drain_merges():
                while merge_q:
                    s_, qb_, part = merge_q.pop(0)
                    L_ = dict(active)[s_]
                    Lq_ = min(128, L_ - qb_ * 128)
                    row = cur[(s_, qb_)]
                    if row[4] is None:
                        row[4] = ("part", part)
                    elif row[4][0] == "part":
                        acc = pairp.tile([128, HPC, 128], DT, tag="pp")
                        nc.vector.tensor_add(acc[:, :, :Lq_], row[4][1], part)
                        row[4] = ("acc", acc)
                    else:
                        acc = row[4][1]
                        nc.vector.tensor_add(acc[:, :, :Lq_], acc[:, :, :Lq_], part)

            def finish_row(s_, qb_):
                L_ = dict(active)[s_]
                Lq_ = min(128, L_ - qb_ * 128)
                row = cur[(s_, qb_)]
                v_sb_ = sbufs[s_][1]
                dp, dLk = row[5]
                emit_pv(row, v_sb_, qb_, dp, dLk, Lq_, stop=True)
                if row[4] is not None:  # tree row: fold diag into acc, one SUM
                    if row[4][0] == "part":
                        acc = pairp.tile([128, HPC, 128], DT, tag="pp")
                        nc.vector.tensor_add(acc[:dLk, :, :Lq_], row[4][1][:dLk], dp)
                        if dLk < 128:
                            nc.vector.tensor_copy(acc[dLk:, :, :Lq_], row[4][1][dLk:])
                        row[4] = ("acc", acc)
                    else:
                        acc = row[4][1]
                        nc.vector.tensor_add(acc[:dLk, :, :Lq_], acc[:dLk, :, :Lq_], dp)
                    sum_q.append((s_, qb_, row[4][1][:, :, :Lq_], 128, Lq_, True))
                else:  # defer row: diag keeps its own ones-matmul
                    sum_q.append((s_, qb_, dp, dLk, Lq_, True))

            for t in range(min(lookahead, len(tasks))):
                emit_S(t)
            for t, (s, L, qb, gi, cg, last) in enumerate(tasks):
                if t + lookahead < len(tasks):
                    emit_S(t + lookahead)
                k_sb, v_sb, q_sb, nqb = sbufs[s]
                style = style_of(qb, nqb)
                Lq = min(128, L - qb * 128)
                st = s_tiles.pop(t)
                pt = ppool.tile([128, GROUP, HPC, 128], DT, tag="p")
                nc.scalar.activation(
                    pt[:, : len(cg), :, :Lq],
                    st[:, : len(cg), :, :Lq],
                    mybir.ActivationFunctionType.Exp,
                    scale=SCALE,
                )
                drain_merges()
                while fin_q:
                    finish_row(*fin_q.pop(0))
                if cg[0] == qb and gi == 0:  # diagonal chunk: causal 0/1 mask
                    meng = nc.vector if qb <= 2 else nc.gpsimd
                    meng.tensor_tensor(
                        pt[:Lq, 0, :, :Lq],
                        pt[:Lq, 0, :, :Lq],
                        mask[:Lq, :, :Lq],
                        mult,
                    )
                while epi_q:
                    epilogue(*epi_q.pop(0))
                if gi == 0:
                    o_ps = opsum.tile([128, HPC, 128], F32, tag="o", name=f"o_{s}_{qb}")
                    sum_ps = smpsum.tile([128, HPC, 128], F32, tag="sm", name=f"sm_{s}_{qb}")
                    cur[(s, qb)] = [o_ps, sum_ps, False, False, None, None]
                row = cur[(s, qb)]
                tree = style == "tree"
                grp_parts = []
                for ci, c in enumerate(cg):
                    Lk = min(128, L - c * 128)
                    p_ap = pt[:Lk, ci, :, :Lq]
                    if c == qb and style != "simple":
                        row[5] = (p_ap, Lk)  # defer diag PV + its sum
                        continue
                    emit_pv(row, v_sb, c, p_ap, Lk, Lq,
                            stop=(style == "simple" and last and ci == len(cg) - 1))
                    if tree:
                        grp_parts.append((p_ap, Lk))
                    else:
                        grp_parts.append((p_ap, Lk))
                if (not tree) and grp_parts:
                    if (style == "defer" and len(grp_parts) == 2
                            and grp_parts[0][1] == 128 and grp_parts[1][1] == 128):
                        pa = pairp.tile([128, HPC, 128], DT, tag="pp")
                        nc.vector.tensor_add(pa[:, :, :Lq], grp_parts[0][0], grp_parts[1][0])
                        sum_q.append((s, qb, pa[:, :, :Lq], 128, Lq, False))
                    else:
                        for pi, (p_ap, Lk) in enumerate(grp_parts):
                            sum_q.append((s, qb, p_ap, Lk, Lq, style == "simple"
                                          and last and pi == len(grp_parts) - 1))
                if tree and grp_parts:
                    if len(grp_parts) == 2 and grp_parts[0][1] == 128 and grp_parts[1][1] == 128:
                        pa = pairp.tile([128, HPC, 128], DT, tag="pp")
                        nc.vector.tensor_add(pa[:, :, :Lq], grp_parts[0][0], grp_parts[1][0])
                        merge_q.append((s, qb, pa[:, :, :Lq]))
                    else:
                        for p_ap, _ in grp_parts:
                            merge_q.append((s, qb, p_ap))
                if last:
                    if style == "simple":
                        pass
                    else:
                        fin_q.append((s, qb))
                drain_sums(1)
            drain_merges()
            while fin_q:
                finish_row(*fin_q.pop(0))
            drain_sums(0)
            while epi_q:
                epilogue(*epi_q.pop(0))
    nc.compile()
    return nc


def get_graph(Ls):
    key = tuple(Ls)
    if key not in _GRAPH_CACHE:
        _GRAPH_CACHE[key] = build_graph(key)
    return _GRAPH_CACHE[key]


def _prep_shards(q, k, v, seqs):
    """Host-side shard + pad + transpose. Returns in_maps for the 8 cores."""
    qb = q.astype(BF16)
    kb = k.astype(BF16)
    vb = v.astype(BF16)
    qp = np.zeros((NUM_SEQS, MAX_SEQLEN, NUM_HEADS, HEAD_DIM), dtype=BF16)
    kp = np.zeros((NUM_SEQS, MAX_SEQLEN, NUM_KV_HEADS, HEAD_DIM), dtype=BF16)
    vp = np.zeros((NUM_SEQS, MAX_SEQLEN, NUM_KV_HEADS, HEAD_DIM), dtype=BF16)
    for s, (st, L) in enumerate(seqs):
        if L:
            qp[s, :L] = qb[st : st + L]
            kp[s, :L] = kb[st : st + L]
            vp[s, :L] = vb[st : st + L]
    in_maps = []
    for i in range(N_CORES):
        hs = slice(HPC * i, HPC * (i + 1))
        qTa = np.ascontiguousarray(qp[:, :, hs, :].transpose(0, 3, 2, 1))
        kTa = np.ascontiguousarray(kp[:, :, i, :].transpose(2, 0, 1))
        vva = np.ascontiguousarray(
            vp[:, :, i, :].reshape(NUM_SEQS, MAX_SEQLEN // 128, 128, HEAD_DIM).transpose(2, 0, 1, 3)
        )
        in_maps.append({"qT": qTa, "kT": kTa, "vv": vva})
    return in_maps


def kernel(q, k, v, cu_seqlens, _trace=False, _tmpdir=None):
    q = np.asarray(q)
    k = np.asarray(k)
    v = np.asarray(v)
    cu = np.asarray(cu_seqlens).astype(np.int64)
    starts = cu[:-1]
    lens = np.clip(cu[1:] - cu[:-1], 0, MAX_SEQLEN)
    seqs = [(int(starts[b]), int(lens[b])) for b in range(NUM_SEQS)]

    out = np.zeros((T_TOTAL, NUM_HEADS, HEAD_DIM), dtype=q.dtype)
    if all(L == 0 for _, L in seqs):
        return out

    nc = get_graph([L for _, L in seqs])
    in_maps = _prep_shards(q, k, v, seqs)
    res = run_bass_kernel_spmd(
        nc,
        in_maps,
        core_ids=list(range(N_CORES)),
        trace=_trace,
        tmpdir=_tmpdir,
    )
    for i in range(N_CORES):
        oT = res.results[i]["out"]  # [128 d, 4 h, s, t] bf16
        o = oT.astype(np.float32).transpose(2, 3, 1, 0)  # [s, t, h, d]
        for s, (st, L) in enumerate(seqs):
            if L:
                out[st : st + L, HPC * i : HPC * (i + 1), :] = o[s, :L]
    if _trace:
        return out, res
    return out



# revision 17
# speedup vs baseline: 1.3647x; 1.0572x over previous
"""Varlen causal GQA flash attention on 8 TRN2 NeuronCores.

Sharding: tensor-parallel over heads. Core i gets Q heads [4i, 4i+4) and
KV head i (GQA group kept intact) -> zero cross-core communication.

Per-core kernel (specialized at build time on the host-visible cu_seqlens):
for each packed sequence (start, L) and each 128-wide query block qb:
  - S^T matmul: lhsT = K^T chunk [128d, <=128 keys], rhs = Q^T [128d, 4h*Lq]
    -> PSUM S^T [keys, (h,q)] (N = 512 streams, bf16).
  - exp on ScalarE straight out of PSUM -> bf16 P^T in SBUF (scale folded in).
  - causal mask on the diagonal chunk: multiply by a 0/1 upper-tri mask (DVE).
  - PV matmuls: lhsT = V chunk [keys, 128d] (natural layout, no P transpose),
    rhs = P^T -> accumulate O^T [128d, 4h*Lq] in PSUM.
  - denominator: all-ones [keys,128] lhsT matmul -> row sums replicated
    across all 128 partitions in PSUM; reciprocal_approx_fast; one
    tensor_tensor multiply normalizes O^T on the way PSUM->SBUF.
  - DMA O^T out; host undoes the transposes (host layout work is free).

The whole core's work is emitted as one flat software pipeline over
(seq, qb, chunk-group) tasks with the S^T matmuls running two groups ahead
of the PV/SUM consumers, so the PE never sits behind exp/mask latency.
"""

import math
import os
import sys

import numpy as np

for _p in ("/opt/trn_rl_repo", "/root/.axon_site/_ro/trn_rl_repo"):
    if os.path.isdir(_p) and _p not in sys.path:
        sys.path.append(_p)

# Under an axon-tunneled container the device run goes through the jax "axon"
# platform; make sure an explicit JAX_PLATFORMS=cpu doesn't hide the devices.
if os.environ.get("TRN_TERMINAL_POOL_IPS") and "jax" not in sys.modules:
    _jp = os.environ.get("JAX_PLATFORMS", "")
    if _jp and "axon" not in _jp:
        os.environ["JAX_PLATFORMS"] = "axon," + _jp

import ml_dtypes

import concourse.bass as bass
import concourse.mybir as mybir
import concourse.tile as tile
from concourse import bacc
from concourse.bass_utils import run_bass_kernel_spmd
from concourse.masks import make_upper_triangular

NUM_HEADS = 32
NUM_KV_HEADS = 8
HEAD_DIM = 128
SCALE = 1.0 / float(np.sqrt(HEAD_DIM))
MAX_SEQLEN = 1024
NUM_SEQS = 4
T_TOTAL = NUM_SEQS * MAX_SEQLEN
N_CORES = 8
HPC = NUM_HEADS // N_CORES  # q heads per core = 4
BF16 = ml_dtypes.bfloat16
GROUP = 2  # key chunks per exp group (PSUM-bank budget bound)

_GRAPH_CACHE = {}


def build_graph(Ls, lookahead=2):
    """Build the SPMD Bass graph, specialized on per-sequence lengths Ls."""
    DT = mybir.dt.bfloat16
    F32 = mybir.dt.float32
    nc = bacc.Bacc(
        "TRN2",
        target_bir_lowering=False,
        debug=False,
        enable_asserts=False,
        num_devices=N_CORES,
    )
    qT = nc.dram_tensor("qT", [NUM_SEQS, 128, HPC, MAX_SEQLEN], DT, kind="ExternalInput")
    kT = nc.dram_tensor("kT", [128, NUM_SEQS, MAX_SEQLEN], DT, kind="ExternalInput")
    vv = nc.dram_tensor("vv", [128, NUM_SEQS, MAX_SEQLEN // 128, 128], DT, kind="ExternalInput")
    outT = nc.dram_tensor("out", [128, HPC, NUM_SEQS, MAX_SEQLEN], DT, kind="ExternalOutput")

    mult = mybir.AluOpType.mult
    active = [(s, L) for s, L in enumerate(Ls) if L > 0]
    nact = len(active)

    with tile.TileContext(nc) as tc:
        with (
            tc.tile_pool(name="consts", bufs=1) as consts,
            tc.tile_pool(name="kin", bufs=nact) as kin,
            tc.tile_pool(name="vin", bufs=nact) as vin,
            tc.tile_pool(name="qin", bufs=nact) as qin,
            tc.tile_pool(name="pt", bufs=4) as ppool,
            tc.tile_pool(name="pairp", bufs=4) as pairp,
            tc.tile_pool(name="osb", bufs=6) as osb,
            tc.tile_pool(name="invp", bufs=3) as invp,
            tc.tile_pool(name="spsum", bufs=2, space="PSUM") as spsum,
            tc.tile_pool(name="opsum", bufs=2, space="PSUM") as opsum,
            tc.tile_pool(name="smpsum", bufs=2, space="PSUM") as smpsum,
        ):
            mask1 = consts.tile([128, 128], DT)
            make_upper_triangular(nc, mask1[:], val=1.0, diag=True)
            # per-head copy keeps the mask multiply off the slow broadcast path
            mask = consts.tile([128, HPC, 128], DT)
            for h in range(HPC):
                nc.vector.tensor_copy(mask[:, h, :], mask1[:])
            ones = consts.tile([128, 128], DT)
            nc.vector.memset(ones[:], 1.0)
            # ---- hoist all input DMAs, in compute order, piecewise (256-col
            # pieces = 512B rows) so each query block's data lands just ahead
            # of its matmuls without queueing behind later sequences' bulk.
            sbufs = {}
            for s, L in active:
                nqb = math.ceil(L / 128)
                k_sb = kin.tile([128, MAX_SEQLEN], DT, tag="k", name=f"k_{s}")
                v_sb = vin.tile([128, MAX_SEQLEN // 128, 128], DT, tag="v", name=f"v_{s}")
                q_sb = qin.tile([128, HPC, MAX_SEQLEN], DT, tag="q", name=f"q_{s}")
                sbufs[s] = (k_sb, v_sb, q_sb, nqb)
            # ALL input DMAs ride the sync queue, finest pieces first, so
            # the scalar queue carries nothing but the act-table warm + the
            # exp stream, and early tasks are never gated on bulk transfers.
            warm = consts.tile([128, 1], F32)
            first = True
            for s, L in active:
                k_sb, v_sb, q_sb, nqb = sbufs[s]
                if first:
                    nc.sync.dma_start(k_sb[:, : min(128, L)], kT[:, s, : min(128, L)])
                    nc.sync.dma_start(q_sb[:, :, : min(128, L)], qT[s, :, :, : min(128, L)])
                    nc.sync.dma_start(v_sb[:, :1, :], vv[:, s, :1, :])
                    # warm the exp table while the first pieces are in flight
                    nc.scalar.activation(
                        warm[:], mask1[:, :1], mybir.ActivationFunctionType.Exp, scale=0.0
                    )
                    if L > 128:
                        nc.sync.dma_start(k_sb[:, 128 : min(256, L)], kT[:, s, 128 : min(256, L)])
                        nc.sync.dma_start(q_sb[:, :, 128 : min(256, L)], qT[s, :, :, 128 : min(256, L)])
                    if L > 256:
                        nc.sync.dma_start(k_sb[:, 256:L], kT[:, s, 256:L])
                        nc.sync.dma_start(q_sb[:, :, 256 : min(384, L)], qT[s, :, :, 256 : min(384, L)])
                    if nqb > 1:
                        nc.sync.dma_start(v_sb[:, 1:nqb, :], vv[:, s, 1:nqb, :])
                    for a, b in ((384, 512), (512, 768), (768, MAX_SEQLEN)):
                        if L > a:
                            nc.sync.dma_start(q_sb[:, :, a : min(b, L)], qT[s, :, :, a : min(b, L)])
                    first = False
                else:
                    nc.sync.dma_start(k_sb[:, :L], kT[:, s, :L])
                    nc.sync.dma_start(q_sb[:, :, : min(512, L)], qT[s, :, :, : min(512, L)])
                    if L > 512:
                        nc.sync.dma_start(q_sb[:, :, 512:L], qT[s, :, :, 512:L])
                    nc.sync.dma_start(v_sb[:, :nqb, :], vv[:, s, :nqb, :])

            # ---- flat task list: one task per (seq, qb, chunk-group)
            # chunks within a qb run diagonal-first (reverse order) so the
            # masked group's DVE latency hides behind later groups.
            tasks = []
            for s, L in active:
                nqb = sbufs[s][3]
                for qb in range(nqb):
                    order = list(range(qb, -1, -1))
                    groups = [order[g : g + GROUP] for g in range(0, len(order), GROUP)]
                    for gi, cg in enumerate(groups):
                        tasks.append((s, L, qb, gi, cg, gi == len(groups) - 1))
            i = 1
            while i < len(tasks):
                if tasks[i][0] != tasks[i - 1][0]:
                    tasks[i - 1], tasks[i] = tasks[i], tasks[i - 1]
                    i += 2
                else:
                    i += 1

            s_tiles = {}

            def emit_S(t):
                s, L, qb, gi, cg, _last = tasks[t]
                _, _, q_sb, _ = sbufs[s]
                k_sb = sbufs[s][0]
                Lq = min(128, L - qb * 128)
                qs = q_sb[:, :, qb * 128 : qb * 128 + Lq]
                st = spsum.tile([128, GROUP, HPC, 128], F32, tag="s")
                s_tiles[t] = st
                for ci, c in enumerate(cg):
                    Lk = min(128, L - c * 128)
                    nc.tensor.matmul(
                        st[:Lk, ci, :, :Lq],
                        lhsT=k_sb[:, c * 128 : c * 128 + Lk],
                        rhs=qs,
                        start=True,
                        stop=True,
                    )

            cur = {}  # per-(s,qb) state: [o_ps, sum_ps, n_pv, sum_started]
            o_tiles = {}
            sum_q = []  # deferred SUM-matmul jobs: (s, qb, rhs_ap, Lk, Lq, last)
            epi_q = []  # epilogues deferred past the next mask (DVE FIFO priority)

            def epilogue(s_, qb_):
                L_ = dict(active)[s_]
                nqb_ = sbufs[s_][3]
                Lq_ = min(128, L_ - qb_ * 128)
                o_ps, sum_ps, _, _ = cur.pop((s_, qb_))
                inv = invp.tile([128, HPC, 128], F32, tag="inv", name=f"inv_{s_}_{qb_}")
                nc.vector.reciprocal_approx_fast(inv[:, :, :Lq_], sum_ps[:, :, :Lq_])
                if qb_ % 2 == 0:
                    o_tiles[s_] = osb.tile([128, HPC, 256], DT, tag="ot", name=f"ot_{s_}_{qb_}")
                o_tile = o_tiles[s_]
                slot = (qb_ % 2) * 128
                nc.vector.tensor_tensor(
                    o_tile[:, :, slot : slot + Lq_], o_ps[:, :, :Lq_], inv[:, :, :Lq_], mult
                )
                if qb_ % 2 == 1 or qb_ == nqb_ - 1:
                    t0 = (qb_ - (qb_ % 2)) * 128
                    w = (qb_ % 2) * 128 + Lq_
                    nc.sync.dma_start(outT[:, :, s_, t0 : t0 + w], o_tile[:, :, :w])

            def drain_sums(keep):
                # emit queued SUM-matmuls, keeping at most `keep` deferred so
                # the GPSIMD pair-add latency stays off the PE FIFO.
                while len(sum_q) > keep:
                    s_, qb_, rhs, Lk_, Lq_, last_ = sum_q.pop(0)
                    st_ = cur[(s_, qb_)]
                    nc.tensor.matmul(
                        st_[1][:, :, :Lq_],
                        lhsT=ones[:Lk_, :],
                        rhs=rhs,
                        start=(not st_[3]),
                        stop=last_,
                    )
                    st_[3] = True
                    if last_:
                        epi_q.append((s_, qb_))

            for t in range(min(lookahead, len(tasks))):
                emit_S(t)
            for t, (s, L, qb, gi, cg, last) in enumerate(tasks):
                if t + lookahead < len(tasks):
                    emit_S(t + lookahead)
                k_sb, v_sb, q_sb, nqb = sbufs[s]
                Lq = min(128, L - qb * 128)
                st = s_tiles.pop(t)
                pt = ppool.tile([128, GROUP, HPC, 128], DT, tag="p")
                nc.scalar.activation(
                    pt[:, : len(cg), :, :Lq],
                    st[:, : len(cg), :, :Lq],
                    mybir.ActivationFunctionType.Exp,
                    scale=SCALE,
                )
                if cg[0] == qb:  # diagonal chunk: causal 0/1 mask
                    nc.vector.tensor_tensor(
                        pt[:Lq, 0, :, :Lq],
                        pt[:Lq, 0, :, :Lq],
                        mask[:Lq, :, :Lq],
                        mult,
                    )
                while epi_q:
                    epilogue(*epi_q.pop(0))
                if gi == 0:
                    o_ps = opsum.tile([128, HPC, 128], F32, tag="o", name=f"o_{s}_{qb}")
                    sum_ps = smpsum.tile([128, HPC, 128], F32, tag="sm", name=f"sm_{s}_{qb}")
                    cur[(s, qb)] = [o_ps, sum_ps, 0, False]
                o_ps, sum_ps, n_pv, _ = cur[(s, qb)]
                for ci, c in enumerate(cg):
                    Lk = min(128, L - c * 128)
                    cur[(s, qb)][2] += 1
                    nc.tensor.matmul(
                        o_ps[:, :, :Lq],
                        lhsT=v_sb[:Lk, c, :],
                        rhs=pt[:Lk, ci, :, :Lq],
                        start=(n_pv + ci == 0),
                        stop=(last and ci == len(cg) - 1),
                    )
                lks = [min(128, L - c * 128) for c in cg]
                # pair-reduce the group's two P chunks on DVE and feed ONE
                # ones-matmul instead of two: trades 226ns of PE (the
                # bottleneck) for ~505ns of spare DVE per group.
                if len(cg) == 2 and qb >= 2 and lks[0] == 128 and lks[1] == 128:
                    pa = pairp.tile([128, HPC, 128], DT, tag="pp")
                    nc.vector.tensor_add(
                        pa[:, :, :Lq], pt[:, 0, :, :Lq], pt[:, 1, :, :Lq]
                    )
                    sum_q.append((s, qb, pa[:, :, :Lq], 128, Lq, last))
                else:
                    for ci, c in enumerate(cg):
                        sum_q.append(
                            (s, qb, pt[: lks[ci], ci, :, :Lq], lks[ci], Lq,
                             last and ci == len(cg) - 1)
                        )
                drain_sums(1)
            drain_sums(0)
            while epi_q:
                epilogue(*epi_q.pop(0))
    nc.compile()
    return nc


def get_graph(Ls):
    key = tuple(Ls)
    if key not in _GRAPH_CACHE:
        _GRAPH_CACHE[key] = build_graph(key)
    return _GRAPH_CACHE[key]


def _prep_shards(q, k, v, seqs):
    """Host-side shard + pad + transpose. Returns in_maps for the 8 cores."""
    qb = q.astype(BF16)
    kb = k.astype(BF16)
    vb = v.astype(BF16)
    qp = np.zeros((NUM_SEQS, MAX_SEQLEN, NUM_HEADS, HEAD_DIM), dtype=BF16)
    kp = np.zeros((NUM_SEQS, MAX_SEQLEN, NUM_KV_HEADS, HEAD_DIM), dtype=BF16)
    vp = np.zeros((NUM_SEQS, MAX_SEQLEN, NUM_KV_HEADS, HEAD_DIM), dtype=BF16)
    for s, (st, L) in enumerate(seqs):
        if L:
            qp[s, :L] = qb[st : st + L]
            kp[s, :L] = kb[st : st + L]
            vp[s, :L] = vb[st : st + L]
    in_maps = []
    for i in range(N_CORES):
        hs = slice(HPC * i, HPC * (i + 1))
        qTa = np.ascontiguousarray(qp[:, :, hs, :].transpose(0, 3, 2, 1))
        kTa = np.ascontiguousarray(kp[:, :, i, :].transpose(2, 0, 1))
        vva = np.ascontiguousarray(
            vp[:, :, i, :].reshape(NUM_SEQS, MAX_SEQLEN // 128, 128, HEAD_DIM).transpose(2, 0, 1, 3)
        )
        in_maps.append({"qT": qTa, "kT": kTa, "vv": vva})
    return in_maps


def kernel(q, k, v, cu_seqlens, _trace=False, _tmpdir=None):
    q = np.asarray(q)
    k = np.asarray(k)
    v = np.asarray(v)
    cu = np.asarray(cu_seqlens).astype(np.int64)
    starts = cu[:-1]
    lens = np.clip(cu[1:] - cu[:-1], 0, MAX_SEQLEN)
    seqs = [(int(starts[b]), int(lens[b])) for b in range(NUM_SEQS)]

    out = np.zeros((T_TOTAL, NUM_HEADS, HEAD_DIM), dtype=q.dtype)
    if all(L == 0 for _, L in seqs):
        return out

    nc = get_graph([L for _, L in seqs])
    in_maps = _prep_shards(q, k, v, seqs)
    res = run_bass_kernel_spmd(
        nc,
        in_maps,
        core_ids=list(range(N_CORES)),
        trace=_trace,
        tmpdir=_tmpdir,
    )
    for i in range(N_CORES):
        oT = res.results[i]["out"]  # [128 d, 4 h, s, t] bf16
        o = oT.astype(np.float32).transpose(2, 3, 1, 0)  # [s, t, h, d]
        for s, (st, L) in enumerate(seqs):
            if L:
                out[st : st + L, HPC * i : HPC * (i + 1), :] = o[s, :L]
    if _trace:
        return out, res
    return out

